# revision 1
# baseline (speedup 1.0000x reference)
"""Trainium2 Bass kernel for the LRU LM (nn_LruLM).

Sharding: trunk is token-sharded (2 batches x 4 seq-chunks of 512 -> 8 cores),
activations kept in transposed [channel, time] layout so every matmul consumes
them as moving rhs with K=channels on partitions. The complex LRU scan is
decomposed into 2 real first-order scans (hardware tensor_tensor_scan) plus
host-precomputed rotation tables:
    a_t = cos(t*th)*xr + sin(t*th)*xi ; b_t = sin(t*th)*xr - cos(t*th)*xi
    A = scan(nu, a) ; B = scan(nu, b)
    hr = cos(t*th)*A + sin(t*th)*B ; hi = sin(t*th)*A - cos(t*th)*B
Cross-chunk scan states are exchanged with a per-layer AllGather over quads.
The final logits matmul is vocab-sharded after an 8-way AllGather of the
final activations. All matmuls run in float32r (~1.5e-4 rel err, full PE rate).
The trunk is processed in two 256-token halves to bound SBUF pressure.
"""

import contextlib

import numpy as np

import concourse.bacc as bacc
import concourse.mybir as mybir
import concourse.tile as tile
from concourse.bass_utils import run_bass_kernel_spmd

AF = mybir.ActivationFunctionType
OP = mybir.AluOpType
F32 = mybir.dt.float32
F32R = mybir.dt.float32r

V, D, L, B, S = 50257, 768, 6, 2, 2048
T = 512                      # tokens per core (trunk chunk)
T2 = 256                     # half-chunk processed at a time
NC = 8
NCHUNK = 4                   # seq chunks per batch
CT = D // 128                # 6 channel tiles
VSH = 6283                   # base vocab shard width
VPAD = 13 * 512              # padded shard width 6656
EPS = 1e-5
QUADS = [[0, 1, 2, 3], [4, 5, 6, 7]]
ALL8 = [list(range(NC))]
# in_proj column-tile order: pair v_r[i] with v_i[i] so rotations can start
# as soon as a pair lands; o tiles afterwards. Host permutes the mt axis.
PERM = [0, 6, 1, 7, 2, 8, 3, 9, 4, 10, 5, 11] + list(range(12, 24))


def _build(nc):
    d = {}
    d["x0t"] = nc.dram_tensor("x0t", [D, T], F32, kind="ExternalInput")
    d["postc"] = nc.dram_tensor("postc", [L, D, T], F32, kind="ExternalInput")
    d["posts"] = nc.dram_tensor("posts", [L, D, T], F32, kind="ExternalInput")
    d["iotat"] = nc.dram_tensor("iotat", [128, T], F32, kind="ExternalInput")
    d["cw"] = nc.dram_tensor("cw", [L, CT, 128, 4], F32, kind="ExternalInput")
    for nm in ["nuv", "lnnu", "gmv", "ln1g", "ln1b", "ln2g", "ln2b", "outbv", "b2v"]:
        d[nm] = nc.dram_tensor(nm, [128, CT * L], F32, kind="ExternalInput")
    for nm in ["lnrg", "lnrb"]:
        d[nm] = nc.dram_tensor(nm, [128, 2 * CT * L], F32, kind="ExternalInput")
    for nm in ["inbv", "b1v"]:
        d[nm] = nc.dram_tensor(nm, [128, 24 * L], F32, kind="ExternalInput")
    for nm in ["lnfg", "lnfb"]:
        d[nm] = nc.dram_tensor(nm, [128, CT], F32, kind="ExternalInput")
    d["w_in"] = nc.dram_tensor("w_in", [L, CT, 128, 24 * 128], F32R, kind="ExternalInput")
    d["w_out"] = nc.dram_tensor("w_out", [L, 2 * CT, 128, CT * 128], F32R, kind="ExternalInput")
    d["w_1"] = nc.dram_tensor("w_1", [L, CT, 128, 24 * 128], F32R, kind="ExternalInput")
    d["w_2"] = nc.dram_tensor("w_2", [L, 24, 128, CT * 128], F32R, kind="ExternalInput")
    d["pwt"] = nc.dram_tensor("pwt", [CT, 13, 128, 512], F32R, kind="ExternalInput")
    d["pbv"] = nc.dram_tensor("pbv", [1, VPAD], F32R, kind="ExternalInput")
    outp = nc.dram_tensor("outp", [B * S, VPAD], F32, kind="ExternalOutput")

    cc_in = [nc.dram_tensor(f"ccin{l}", [128, 2 * CT], F32) for l in range(L)]
    cc_out = [
        nc.dram_tensor(f"ccout{l}", [NCHUNK * 128, 2 * CT], F32)
        for l in range(L)
    ]
    xf_in = nc.dram_tensor("xfin", [D, T], F32R)
    xf_all = nc.dram_tensor("xfall", [NC * D, T], F32R, addr_space="Shared")

    HS = (slice(0, T2), slice(T2, T))

    with tile.TileContext(nc) as tc:
        est = contextlib.ExitStack()
        with est:
            vec = est.enter_context(tc.tile_pool(name="vec", bufs=1))
            rowp = est.enter_context(tc.tile_pool(name="rowp", bufs=8))
            tmp3 = est.enter_context(tc.tile_pool(name="tmp3", bufs=3))
            tmp4 = est.enter_context(tc.tile_pool(name="tmp4", bufs=4))
            scal = est.enter_context(tc.tile_pool(name="scal", bufs=8))
            smal = est.enter_context(tc.tile_pool(name="smal", bufs=2))
            ps_sm = est.enter_context(tc.tile_pool(name="pssm", bufs=1, space="PSUM"))
            ps_bc = est.enter_context(tc.tile_pool(name="psbc", bufs=1, space="PSUM"))
            ps_mm = est.enter_context(tc.tile_pool(name="psmm", bufs=4, space="PSUM"))

            ones128 = vec.tile([128, 1], F32, tag="ones128")
            nc.vector.memset(ones128[:], 1.0)
            onesrow = vec.tile([1, 128], F32, tag="onesrow")
            nc.vector.memset(onesrow[:], 1.0)
            onesrow_r = vec.tile([1, 128], F32R, tag="onesrowr")
            nc.vector.tensor_copy(onesrow_r[:], onesrow[:])
            epst = vec.tile([1, 1], F32, tag="epst")
            nc.vector.memset(epst[:], EPS)
            iota_t = vec.tile([128, T], F32, tag="iota")
            nc.sync.dma_start(iota_t[:], d["iotat"][:])

            vt = {}
            for nm in ["nuv", "lnnu", "gmv", "ln1g", "ln1b", "ln2g", "ln2b",
                       "outbv", "b2v", "lnrg", "lnrb", "inbv", "b1v", "lnfg", "lnfb"]:
                vt[nm] = vec.tile(list(d[nm].shape), F32, tag=nm, name=nm)
                nc.sync.dma_start(vt[nm][:], d[nm][:])

            def layer_norm(xaps, g_ap, b_ap, out_pool, out_dtype, out_tag):
                """LN over channels (partition dim, across len(xaps) [128,T2] APs)."""
                n = len(xaps)
                nch = float(n * 128)
                ps_s = ps_sm.tile([1, T2], F32, tag="ps_s")
                ps_q = ps_sm.tile([1, T2], F32, tag="ps_q")
                for i in range(n):
                    nc.tensor.matmul(ps_s[:], ones128[:], xaps[i],
                                     start=(i == 0), stop=(i == n - 1))
                for i in range(n):
                    sq = tmp3.tile([128, T2], F32, tag="sqt")
                    nc.scalar.activation(sq[:], xaps[i], AF.Square)
                    nc.tensor.matmul(ps_q[:], ones128[:], sq[:],
                                     start=(i == 0), stop=(i == n - 1))
                m = rowp.tile([1, T2], F32, tag="lnrow")
                nc.vector.tensor_scalar_mul(m[:], ps_s[:], 1.0 / nch)
                mq = rowp.tile([1, T2], F32, tag="lnrow")
                nc.vector.tensor_scalar_mul(mq[:], ps_q[:], 1.0 / nch)
                var = rowp.tile([1, T2], F32, tag="lnrow")
                nc.vector.scalar_tensor_tensor(var[:], m[:], -1.0, m[:],
                                               OP.mult, OP.mult)
                nc.vector.tensor_tensor(var[:], mq[:], var[:], OP.add)
                sd = rowp.tile([1, T2], F32, tag="lnrow")
                nc.scalar.activation(sd[:], var[:], AF.Sqrt, bias=epst[:])
                rstd = rowp.tile([1, T2], F32, tag="lnrow")
                nc.vector.reciprocal(rstd[:], sd[:])
                a0 = rowp.tile([1, T2], F32, tag="lnrow")
                nc.vector.scalar_tensor_tensor(a0[:], m[:], -1.0, rstd[:],
                                               OP.mult, OP.mult)
                bs = ps_bc.tile([128, T2], F32, tag="bc_s")
                nc.tensor.matmul(bs[:], onesrow[:], rstd[:], start=True, stop=True)
                ba = ps_bc.tile([128, T2], F32, tag="bc_a")
                nc.tensor.matmul(ba[:], onesrow[:], a0[:], start=True, stop=True)
                outs = []
                for i in range(n):
                    t1 = tmp3.tile([128, T2], F32, tag="lnt1")
                    nc.vector.tensor_tensor(t1[:], xaps[i], bs[:], OP.mult)
                    nc.vector.tensor_tensor(t1[:], t1[:], ba[:], OP.add)
                    o = out_pool.tile([128, T2], out_dtype, tag=out_tag)
                    nc.vector.tensor_scalar(o[:], t1[:], g_ap(i), b_ap(i),
                                            OP.mult, OP.add)
                    outs.append(o)
                return outs

            def proj(w_dram, l, kts, n_mt, m_base, rhs_pos, grp, out_cb):
                """psum[(mt,h)] = sum_kt w[l,kt,m_base+mt].T @ rhs.
                kts: dram kt indices; rhs_pos[h][ki]: rhs tile for position ki.
                out_cb(mt, h, psum)."""
                for m0 in range(0, n_mt, grp):
                    g = min(grp, n_mt - m0)
                    psl = {(mi, h): ps_mm.tile([128, T2], F32, tag="mmps",
                                               name="mmps")
                           for mi in range(g) for h in range(2)}
                    for ki, kt in enumerate(kts):
                        wt = wstrm.tile([128, g * 128], F32R, tag="wstrm")
                        nc.sync.dma_start(
                            wt[:],
                            w_dram[l, kt, :,
                                   (m_base + m0) * 128:(m_base + m0 + g) * 128],
                        )
                        for mi in range(g):
                            for h in range(2):
                                nc.tensor.matmul(
                                    psl[(mi, h)][:],
                                    wt[:, mi * 128:(mi + 1) * 128],
                                    rhs_pos[h][ki][:],
                                    start=(ki == 0), stop=(ki == len(kts) - 1),
                                )
                    for mi in range(g):
                        for h in range(2):
                            out_cb(m0 + mi, h, psl[(mi, h)])

            tst = contextlib.ExitStack()
            with tst:
                xres = tst.enter_context(tc.tile_pool(name="xres", bufs=7))
                rhs = tst.enter_context(tc.tile_pool(name="rhs", bufs=25))
                tabs = tst.enter_context(tc.tile_pool(name="tabs", bufs=7))
                wstrm = tst.enter_context(tc.tile_pool(name="wstrm", bufs=4))
                uvp = tst.enter_context(tc.tile_pool(name="uvp", bufs=6))
                abp = tst.enter_context(tc.tile_pool(name="abp", bufs=3))
                ABp = tst.enter_context(tc.tile_pool(name="ABp", bufs=13))
                gg = tst.enter_context(tc.tile_pool(name="gg", bufs=24))
                ypp = tst.enter_context(tc.tile_pool(name="ypp", bufs=13))

                x = []
                for i in range(CT):
                    xt = xres.tile([128, T], F32, tag="x")
                    nc.sync.dma_start(xt[:], d["x0t"][i * 128:(i + 1) * 128, :])
                    x.append(xt)

                for l in range(L):
                    co = l * CT
                    co2 = l * 2 * CT
                    pc, psn = [], []
                    for i in range(CT):
                        t1 = tabs.tile([128, T], F32, tag="postc")
                        nc.sync.dma_start(t1[:], d["postc"][l, i * 128:(i + 1) * 128, :])
                        pc.append(t1)
                        t2 = tabs.tile([128, T], F32, tag="posts")
                        nc.sync.dma_start(t2[:], d["posts"][l, i * 128:(i + 1) * 128, :])
                        psn.append(t2)

                    ln1 = [layer_norm(
                        [x[i][:, HS[h]] for i in range(CT)],
                        lambda i: vt["ln1g"][:, co + i:co + i + 1],
                        lambda i: vt["ln1b"][:, co + i:co + i + 1],
                        rhs, F32R, "lnout") for h in range(2)]

                    vtl, so, AB = {}, {}, {}

                    def rot_scan(i, l=l, co=co, pc=pc, psn=psn, vtl=vtl, AB=AB):
                        gcol = vt["gmv"][:, co + i:co + i + 1]
                        nub = abp.tile([128, T2], F32, tag="nub")
                        nc.vector.tensor_scalar(
                            nub[:], iota_t[:, :T2], 0.0,
                            vt["nuv"][:, co + i:co + i + 1], OP.mult, OP.add)
                        prevA = prevB = None
                        for h in range(2):
                            sl = HS[h]
                            prc = abp.tile([128, T2], F32, tag="abtmp", bufs=6, name="prec")
                            nc.vector.tensor_scalar_mul(prc[:], pc[i][:, sl], gcol)
                            prs = abp.tile([128, T2], F32, tag="abtmp", bufs=6, name="pres")
                            nc.vector.tensor_scalar_mul(prs[:], psn[i][:, sl], gcol)
                            t1 = abp.tile([128, T2], F32, tag="abtmp", bufs=6, name="rt1")
                            nc.vector.tensor_tensor(t1[:], prc[:], vtl[(i, h)][:], OP.mult)
                            t2 = abp.tile([128, T2], F32, tag="abtmp", bufs=6, name="rt2")
                            nc.vector.tensor_tensor(t2[:], prs[:], vtl[(CT + i, h)][:], OP.mult)
                            av = abp.tile([128, T2], F32, tag="av")
                            nc.vector.tensor_tensor(av[:], t1[:], t2[:], OP.add)
                            t3 = abp.tile([128, T2], F32, tag="abtmp", bufs=6, name="rt1")
                            nc.vector.tensor_tensor(t3[:], prs[:], vtl[(i, h)][:], OP.mult)
                            t4 = abp.tile([128, T2], F32, tag="abtmp", bufs=6, name="rt2")
                            nc.vector.tensor_tensor(t4[:], prc[:], vtl[(CT + i, h)][:], OP.mult)
                            bv = abp.tile([128, T2], F32, tag="bv")
                            nc.vector.tensor_tensor(bv[:], t3[:], t4[:], OP.subtract)
                            Av = ABp.tile([128, T2], F32, tag="Av")
                            nc.vector.tensor_tensor_scan(
                                Av[:], nub[:], av[:],
                                0.0 if h == 0 else prevA[:, T2 - 1:T2],
                                OP.mult, OP.add)
                            Bv = ABp.tile([128, T2], F32, tag="Bv")
                            nc.vector.tensor_tensor_scan(
                                Bv[:], nub[:], bv[:],
                                0.0 if h == 0 else prevB[:, T2 - 1:T2],
                                OP.mult, OP.add)
                            AB[(i, h, 0)] = Av
                            AB[(i, h, 1)] = Bv
                            prevA, prevB = Av, Bv

                    def in_cb(mpos, h, ps, l=l, vtl=vtl, so=so, rot_scan=rot_scan):
                        mt = PERM[mpos]
                        bias = vt["inbv"][:, l * 24 + mt:l * 24 + mt + 1]
                        if mt < 2 * CT:
                            vv = uvp.tile([128, T2], F32, tag="vtile")
                            nc.scalar.activation(vv[:], ps[:], AF.Identity, bias=bias)
                            vtl[(mt, h)] = vv
                            if mt >= CT and h == 1:
                                rot_scan(mt - CT)
                        else:
                            s = gg.tile([128, T2], F32, tag="gate_gelu")
                            nc.scalar.activation(s[:], ps[:], AF.Silu, bias=bias)
                            so[(mt - 2 * CT, h)] = s

                    proj(d["w_in"], l, list(range(CT)), 24, 0, ln1, 2, in_cb)

                    # exchange boundary states within quad
                    endAB = smal.tile([128, 2 * CT], F32, tag="endAB")
                    for i in range(CT):
                        nc.vector.tensor_copy(endAB[:, i:i + 1],
                                              AB[(i, 1, 0)][:, T2 - 1:T2])
                        nc.vector.tensor_copy(endAB[:, CT + i:CT + i + 1],
                                              AB[(i, 1, 1)][:, T2 - 1:T2])
                    nc.sync.dma_start(cc_in[l][:], endAB[:])
                    nc.gpsimd.collective_compute(
                        "AllGather", OP.bypass, replica_groups=QUADS,
                        ins=[cc_in[l][:]], outs=[cc_out[l][:]],
                    )
                    gat = smal.tile([128, NCHUNK * 2 * CT], F32, tag="gat")
                    nc.sync.dma_start(
                        gat[:].rearrange("p (j c) -> p j c", c=2 * CT),
                        cc_out[l][:].rearrange("(j p) c -> p j c", p=128),
                    )

                    # compose prefix states (shared by both halves)
                    inis = {}
                    for i in range(CT):
                        cwt = scal.tile([128, 4], F32, tag="cwt")
                        nc.sync.dma_start(cwt[:], d["cw"][l, i])
                        for ab, base in ((0, 0), (1, CT)):
                            ini = scal.tile([128, 1], F32, tag="ini", bufs=4)
                            nc.vector.tensor_scalar_mul(
                                ini[:], gat[:, base + i:base + i + 1], cwt[:, 0:1])
                            for j in (1, 2):
                                tg, bf = (("inifin", 13) if j == 2 else ("ini", 4))
                                ini2 = scal.tile([128, 1], F32, tag=tg, bufs=bf,
                                                 name="ini2")
                                nc.vector.scalar_tensor_tensor(
                                    ini2[:],
                                    gat[:, j * 2 * CT + base + i:
                                        j * 2 * CT + base + i + 1],
                                    cwt[:, j:j + 1], ini[:], OP.mult, OP.add)
                                ini = ini2
                            inis[(i, ab)] = ini

                    # correct scans, post-rotate, gate, LNr (per half)
                    yn = [None, None]
                    for h in range(2):
                        sl = HS[h]
                        ys = [None] * (2 * CT)
                        for i in range(CT):
                            npw = tmp4.tile([128, T2], F32, tag="npw")
                            nc.scalar.activation(
                                npw[:], iota_t[:, sl], AF.Exp,
                                scale=vt["lnnu"][:, co + i:co + i + 1])
                            corr = []
                            for ab in (0, 1):
                                Sc = tmp4.tile([128, T2], F32, tag="Sc")
                                nc.vector.scalar_tensor_tensor(
                                    Sc[:], npw[:], inis[(i, ab)][:],
                                    AB[(i, h, ab)][:], OP.mult, OP.add)
                                corr.append(Sc)
                            t1 = abp.tile([128, T2], F32, tag="abtmp", bufs=6, name="rt1")
                            nc.vector.tensor_tensor(t1[:], pc[i][:, sl], corr[0][:], OP.mult)
                            t2 = abp.tile([128, T2], F32, tag="abtmp", bufs=6, name="rt2")
                            nc.vector.tensor_tensor(t2[:], psn[i][:, sl], corr[1][:], OP.mult)
                            hr = abp.tile([128, T2], F32, tag="av")
                            nc.vector.tensor_tensor(hr[:], t1[:], t2[:], OP.add)
                            yv = ypp.tile([128, T2], F32, tag="y_part")
                            nc.vector.tensor_tensor(yv[:], hr[:], so[(i, h)][:], OP.mult)
                            ys[i] = yv
                            t3 = abp.tile([128, T2], F32, tag="abtmp", bufs=6, name="rt1")
                            nc.vector.tensor_tensor(t3[:], psn[i][:, sl], corr[0][:], OP.mult)
                            t4 = abp.tile([128, T2], F32, tag="abtmp", bufs=6, name="rt2")
                            nc.vector.tensor_tensor(t4[:], pc[i][:, sl], corr[1][:], OP.mult)
                            hi = abp.tile([128, T2], F32, tag="bv")
                            nc.vector.tensor_tensor(hi[:], t3[:], t4[:], OP.subtract)
                            yv2 = ypp.tile([128, T2], F32, tag="y_part")
                            nc.vector.tensor_tensor(yv2[:], hi[:], so[(CT + i, h)][:], OP.mult)
                            ys[CT + i] = yv2
                        yn[h] = layer_norm(
                            [t[:] for t in ys],
                            lambda i: vt["lnrg"][:, co2 + i:co2 + i + 1],
                            lambda i: vt["lnrb"][:, co2 + i:co2 + i + 1],
                            rhs, F32R, "lnout")

                    # out_proj + residual
                    xn = {}

                    def out_cb(mt, h, ps, l=l, co=co, xn=xn):
                        po = tmp3.tile([128, T2], F32, tag="po")
                        nc.scalar.activation(po[:], ps[:], AF.Identity,
                                             bias=vt["outbv"][:, co + mt:co + mt + 1])
                        if mt not in xn:
                            xn[mt] = xres.tile([128, T], F32, tag="x", name="xn")
                        nc.vector.tensor_tensor(xn[mt][:, HS[h]], x[mt][:, HS[h]],
                                                po[:], OP.add)

                    proj(d["w_out"], l, list(range(2 * CT)), CT, 0, yn, 2, out_cb)
                    for i in range(CT):
                        x[i] = xn[i]

                    ln2 = [layer_norm(
                        [x[i][:, HS[h]] for i in range(CT)],
                        lambda i: vt["ln2g"][:, co + i:co + i + 1],
                        lambda i: vt["ln2b"][:, co + i:co + i + 1],
                        rhs, F32R, "lnout") for h in range(2)]

                    # MLP with hidden dim split in two passes to bound SBUF
                    xn2 = {}
                    part = {}
                    for p in range(2):
                        gl = {}

                        def mlp1_cb(mt, h, ps, l=l, p=p, gl=gl):
                            mt_abs = p * 12 + mt
                            g = gg.tile([128, T2], F32R, tag="gate_gelu")
                            nc.scalar.activation(
                                g[:], ps[:], AF.Gelu,
                                bias=vt["b1v"][:, l * 24 + mt_abs:l * 24 + mt_abs + 1])
                            gl[(mt, h)] = g

                        proj(d["w_1"], l, list(range(CT)), 12, p * 12, ln2, 2, mlp1_cb)
                        grhs = [[gl[(j, h)] for j in range(12)] for h in range(2)]

                        def mlp2_cb(mt, h, ps, l=l, co=co, p=p, part=part, xn2=xn2):
                            if p == 0:
                                pt = ypp.tile([128, T2], F32, tag="y_part")
                                nc.scalar.activation(pt[:], ps[:], AF.Identity)
                                part[(mt, h)] = pt
                            else:
                                po = tmp3.tile([128, T2], F32, tag="po")
                                nc.scalar.activation(
                                    po[:], ps[:], AF.Identity,
                                    bias=vt["b2v"][:, co + mt:co + mt + 1])
                                nc.vector.tensor_tensor(po[:], po[:],
                                                        part[(mt, h)][:], OP.add)
                                if mt not in xn2:
                                    xn2[mt] = xres.tile([128, T], F32, tag="x", name="xn2")
                                nc.vector.tensor_tensor(
                                    xn2[mt][:, HS[h]], x[mt][:, HS[h]], po[:], OP.add)

                        proj(d["w_2"], l, list(range(p * 12, p * 12 + 12)), CT, 0,
                             grhs, 2, mlp2_cb)
                    for i in range(CT):
                        x[i] = xn2[i]

                # final LN -> xf (f32r) -> gather across all 8 cores
                for h in range(2):
                    xf = layer_norm(
                        [x[i][:, HS[h]] for i in range(CT)],
                        lambda i: vt["lnfg"][:, i:i + 1],
                        lambda i: vt["lnfb"][:, i:i + 1],
                        rhs, F32R, "lnout")
                    for i in range(CT):
                        nc.sync.dma_start(
                            xf_in[i * 128:(i + 1) * 128, HS[h]], xf[i][:])
                nc.gpsimd.collective_compute(
                    "AllGather", OP.bypass, replica_groups=ALL8,
                    ins=[xf_in[:]], outs=[xf_all[:]],
                )

            # ---------------- logits phase ----------------
            lst = contextlib.ExitStack()
            with lst:
                pwp = lst.enter_context(tc.tile_pool(name="pwp", bufs=44))
                xfp = lst.enter_context(tc.tile_pool(name="xfp", bufs=12))
                outp_p = lst.enter_context(tc.tile_pool(name="outpp", bufs=8))
                pbt = outp_p.tile([1, VPAD], F32R, tag="pbt", bufs=1)
                nc.sync.dma_start(pbt[:], d["pbv"][:])

                cnt = 0
                for v0, v1 in ((0, 7), (7, 13)):
                    pwtl = {}
                    for kt in range(CT):
                        for vn in range(v0, v1):
                            w = pwp.tile([128, 512], F32R, tag="pw")
                            nc.sync.dma_start(w[:], d["pwt"][kt, vn])
                            pwtl[(kt, vn)] = w
                    for tb in range(NC):
                        xfb = []
                        for kt in range(CT):
                            xt = xfp.tile([128, T], F32R, tag="xfb")
                            nc.sync.dma_start(
                                xt[:],
                                xf_all[tb * D + kt * 128: tb * D + (kt + 1) * 128, :],
                            )
                            xfb.append(xt)
                        for mt in range(4):
                            for vn in range(v0, v1):
                                ps = ps_mm.tile([128, 512], F32, tag="mmps")
                                for kt in range(CT):
                                    nc.tensor.matmul(
                                        ps[:],
                                        xfb[kt][:, mt * 128:(mt + 1) * 128],
                                        pwtl[(kt, vn)][:],
                                        start=(kt == 0), stop=False,
                                    )
                                nc.tensor.matmul(
                                    ps[:], onesrow_r[:],
                                    pbt[:, vn * 512:(vn + 1) * 512],
                                    start=False, stop=True,
                                )
                                ot = outp_p.tile([128, 512], F32, tag="ot")
                                if cnt % 2 == 0:
                                    nc.scalar.activation(ot[:], ps[:], AF.Copy)
                                else:
                                    nc.vector.tensor_copy(ot[:], ps[:])
                                cnt += 1
                                r0 = tb * T + mt * 128
                                nc.sync.dma_start(
                                    outp[r0:r0 + 128, vn * 512:(vn + 1) * 512], ot[:]
                                )
    return d


def _host_prep(inputs):
    f32 = np.float32
    tokens = np.asarray(inputs["tokens"]).astype(np.int64)
    emb = np.asarray(inputs["emb"], dtype=f32)
    theta = np.exp(np.asarray(inputs["theta_log"], dtype=np.float64))
    nu = np.exp(-np.exp(np.asarray(inputs["nu_log"], dtype=np.float64)))
    gamma = np.exp(np.asarray(inputs["gamma_log"], dtype=np.float64))

    def vec_tile(a, per_l):
        a = np.asarray(a, dtype=f32)
        if a.ndim == 1:
            a = a[None, :]
        Ln = a.shape[0]
        out = np.zeros((128, per_l * Ln), f32)
        for l in range(Ln):
            out[:, l * per_l:(l + 1) * per_l] = a[l].reshape(per_l, 128).T
        return out

    def mm_tile(w, ktn, perm=None):
        w = np.asarray(w, dtype=f32)
        Ln, K, M = w.shape
        out = w.reshape(Ln, ktn, 128, M)
        if perm is not None:
            mt = M // 128
            out = out.reshape(Ln, ktn, 128, mt, 128)[:, :, :, perm, :]
            out = out.reshape(Ln, ktn, 128, M)
        return np.ascontiguousarray(out)

    base = {
        "iotat": np.broadcast_to(np.arange(1, T + 1, dtype=f32), (128, T)).copy(),
        "nuv": vec_tile(nu.astype(f32), CT),
        "lnnu": vec_tile(np.log(nu).astype(f32), CT),
        "gmv": vec_tile(gamma.astype(f32), CT),
        "ln1g": vec_tile(inputs["ln1_g"], CT),
        "ln1b": vec_tile(inputs["ln1_b"], CT),
        "ln2g": vec_tile(inputs["ln2_g"], CT),
        "ln2b": vec_tile(inputs["ln2_b"], CT),
        "outbv": vec_tile(inputs["outb"], CT),
        "b2v": vec_tile(inputs["b2"], CT),
        "lnrg": vec_tile(inputs["lnr_g"], 2 * CT),
        "lnrb": vec_tile(inputs["lnr_b"], 2 * CT),
        "inbv": vec_tile(inputs["inb"], 24),
        "b1v": vec_tile(inputs["b1"], 24),
        "lnfg": vec_tile(inputs["lnf_g"], CT),
        "lnfb": vec_tile(inputs["lnf_b"], CT),
        "w_in": mm_tile(inputs["inw"], CT, perm=PERM),
        "w_out": mm_tile(inputs["outw"], 2 * CT),
        "w_1": mm_tile(inputs["w1"], CT),
        "w_2": mm_tile(inputs["w2"], 24),
    }

    pw = np.asarray(inputs["pw"], dtype=f32)
    pb = np.asarray(inputs["pb"], dtype=f32)
    tok_flat = tokens.reshape(-1)

    in_maps = []
    t_loc = np.arange(T, dtype=np.float64)
    for k in range(NC):
        m = k % NCHUNK
        off = m * T
        rows = tok_flat[k * T:(k + 1) * T]
        x0t = np.ascontiguousarray(emb[rows].T.astype(f32))
        tg = off + t_loc
        ang = tg[None, None, :] * theta[:, :, None]
        postc = np.cos(ang).astype(f32)
        posts = np.sin(ang).astype(f32)
        cw = np.zeros((L, CT, 128, 4), f32)
        for j in range(m):
            wj = nu ** (T * (m - 1 - j))
            cw[:, :, :, j] = wj.reshape(L, CT, 128).astype(f32)
        vs = min(VSH * k, V)
        ve = min(vs + VSH, V)
        pwk = np.zeros((D, VPAD), f32)
        pwk[:, :ve - vs] = pw[:, vs:ve]
        pbk = np.zeros((1, VPAD), f32)
        pbk[0, :ve - vs] = pb[vs:ve]
        pwt = np.ascontiguousarray(
            pwk.reshape(CT, 128, 13, 512).transpose(0, 2, 1, 3))
        mm = dict(base)
        mm.update({"x0t": x0t, "postc": postc, "posts": posts,
                   "cw": cw, "pwt": pwt, "pbv": pbk})
        in_maps.append(mm)
    return in_maps


_CACHE = {}


def _get_nc():
    if "nc" not in _CACHE:
        nc = bacc.Bacc("TRN2", target_bir_lowering=False, debug=False,
                       num_devices=NC)
        _build(nc)
        nc.compile()
        _CACHE["nc"] = nc
    return _CACHE["nc"]


def kernel(**inputs):
    nc = _get_nc()
    in_maps = _host_prep(inputs)
    res = run_bass_kernel_spmd(nc, in_maps, core_ids=list(range(NC)),
                               trace=False)
    out = np.empty((B * S, V), np.float32)
    for k in range(NC):
        vs = min(VSH * k, V)
        ve = min(vs + VSH, V)
        out[:, vs:ve] = res.results[k]["outp"][:, :ve - vs]
    return out.reshape(B, S, V)

